# revision 1
# baseline (speedup 1.0000x reference)
"""Trainium2 Bass kernel for nn_ConsciousnessMonitor (histogram_binning).

kernel(**inputs) takes FULL unsharded numpy inputs, returns the full (9,)
float32 output. Shards state_history along time across 8 NeuronCores:
masked means via fp32r PE matmul while streaming (DMA-bound), min/max via
PE-transpose combine + AllReduce(max), packed joint histograms via one
[40,40] PSUM accumulation + AllReduce(add), MI tail vectorized across the
4 partitions; differentiation branch scheduled into the collective tail.

Self-contained: shapes/sharding hardcoded; reads no sibling files.
"""
import numpy as np

import concourse.bacc as bacc
import concourse.tile as tile
import concourse.mybir as mybir
from concourse.bass_utils import run_bass_kernel_spmd
from concourse.masks import make_identity
import concourse.bass_isa as bass_isa

F32 = mybir.dt.float32
F32R = mybir.dt.float32r
BF16 = mybir.dt.bfloat16
I32 = mybir.dt.int32
AX = mybir.AxisListType
OP = mybir.AluOpType
ACT = mybir.ActivationFunctionType

N_CORES = 8
T, D = 32768, 2048
TL = T // N_CORES          # 4096 time steps per core
NB = 10                    # histogram bins per axis
NPAIR = 4                  # partitions (mask pairs)
J = 2 * NPAIR              # 8 masked-mean columns
NTC = TL // 512            # 8 accumulator groups (512 t each)
NDC = D // 128             # 16 contraction chunks
NCH = TL // 128            # 32 binning chunks of 128 t
MEM = 100
SN = 10
TINV = float(1.0 / (np.float32(T) + np.float32(1e-10)))

# accumulator tcn -> (bank b, quadrant q): tcn = 3*b + q, q in {0,1,2}
ACC_MAP = [(tcn // 3, tcn % 3) for tcn in range(NTC)]

_CACHE = {}
LAST_RESULTS = None


def _steer_act_tables():
    """Make ln/exp resolve to the combined natural_log_exp_and_others set
    so the tail needs a single activation-table load instead of thrashing
    between the ln-only and exp-only sets. Set ids are preserved; restore
    after build."""
    import concourse.bacc as bacc_mod
    orig = bacc_mod.get_activation_tables

    def patched(arch):
        t = orig(arch)
        if "natural_log_exp_and_others" in t:
            for name, funcs in t.items():
                if name != "natural_log_exp_and_others":
                    funcs.discard(ACT.Ln)
                    funcs.discard(ACT.Exp)
        return t

    bacc_mod.get_activation_tables = patched
    return lambda: setattr(bacc_mod, "get_activation_tables", orig)


STEER_ACT = False


def _build(debug=False, variant="main"):
    restore = _steer_act_tables() if STEER_ACT else (lambda: None)
    try:
        return _build_inner(debug, variant)
    finally:
        restore()


def _build_inner(debug, variant):
    sim1 = variant.startswith("sim1")
    nc = bacc.Bacc("TRN2", target_bir_lowering=False, debug=False,
                   num_devices=1 if sim1 else N_CORES)
    ht = nc.dram_tensor("ht", [D, TL], F32R, kind="ExternalInput").ap()
    msb = nc.dram_tensor("msb", [128, NDC * J], F32R,
                         kind="ExternalInput").ap()
    invc2 = nc.dram_tensor("invc2", [1, 2 * J], F32,
                           kind="ExternalInput").ap()
    memsb = nc.dram_tensor("memsb", [128, NDC * MEM], F32,
                           kind="ExternalInput").ap()
    sampsb = nc.dram_tensor("sampsb", [128, NDC * SN], F32,
                            kind="ExternalInput").ap()
    selc = nc.dram_tensor("selc", [NPAIR * NB, NPAIR], F32,
                          kind="ExternalInput").ap()
    selcT = nc.dram_tensor("selcT", [NPAIR, NPAIR * NB], F32,
                           kind="ExternalInput").ap()
    out = nc.dram_tensor("out", [9], F32, kind="ExternalOutput").ap()

    rg = [list(range(N_CORES))]

    with tile.TileContext(nc) as tc:
        with tc.tile_pool(name="consts", bufs=1) as consts, \
             tc.tile_pool(name="sb", bufs=1) as sb, \
             tc.tile_pool(name="htp", bufs=3) as htp, \
             tc.tile_pool(name="dram", bufs=1, space="DRAM") as dram:

            # ---- small input DMAs needed for streaming (gpsimd queue) ----
            m_sb = consts.tile([128, NDC * J], F32R, tag="msb")
            nc.gpsimd.dma_start(out=m_sb[:], in_=msb[:])
            invc2_sb = consts.tile([1, 2 * J], F32, tag="invc2")
            nc.gpsimd.dma_start(out=invc2_sb[:], in_=invc2[:])
            selc_sb = consts.tile([NPAIR * NB, NPAIR], F32, tag="selc")
            nc.gpsimd.dma_start(out=selc_sb[:], in_=selc[:])
            selcT_sb = consts.tile([NPAIR, NPAIR * NB], F32, tag="selct")
            nc.gpsimd.dma_start(out=selcT_sb[:], in_=selcT[:])

            # ---- constants ----
            ident10 = consts.tile([NB, NB], F32, tag="id10")
            make_identity(nc, ident10[:])
            ones128 = consts.tile([128, 1], F32, tag="o128")
            nc.gpsimd.memset(ones128[:], 1.0)
            ones10 = consts.tile([NB, 1], F32, tag="o10")
            nc.gpsimd.memset(ones10[:], 1.0)
            ones1_10 = consts.tile([1, NB], F32, tag="o110")
            nc.gpsimd.memset(ones1_10[:], 1.0)
            iota10 = consts.tile([128, NB], I32, tag="iota10")
            nc.gpsimd.iota(iota10[:], pattern=[[1, NB]], base=0,
                           channel_multiplier=0)

            # stS rows 0..7 = raw S.T (t-ordered cols); row 8 = ones
            stS = sb.tile([J + 1, TL], F32, tag="sts")
            ones4096 = sb.tile([1, TL], F32, tag="ones4096")
            nc.gpsimd.memset(ones4096[:], 1.0)
            nc.gpsimd.dma_start(out=stS[J:J + 1, :], in_=ones4096[:])

            # preload the ln+exp activation table (set 6) off the
            # critical path; the whole tail uses only ln/exp
            lnpre = sb.tile([1, 1], F32, tag="lnpre")
            nc.vector.memset(lnpre[:], 1.0)
            nc.scalar.activation(lnpre[:], lnpre[:], ACT.Ln)

            # ---- stage A: stream HT, S.T = M.T @ HT (fp32r, DMA-bound;
            # fp32r matmul outputs must start at partition 0) ----
            psa_pool = tc.tile_pool(name="psa", bufs=8, space="PSUM")
            psa_ctx = psa_pool.__enter__()
            psA = [psa_ctx.tile([J, 512], F32, tag="psa", name=f"psA{g}")
                   for g in range(NTC)]
            for dk in range(NDC):
                htt = htp.tile([128, TL], F32R, tag="htt", name="htt")
                nc.sync.dma_start(out=htt[:],
                                  in_=ht[dk * 128:(dk + 1) * 128, :])
                for tcn in range(NTC):
                    nc.tensor.matmul(
                        psA[tcn][:],
                        m_sb[:, dk * J:(dk + 1) * J],
                        htt[:, tcn * 512:(tcn + 1) * 512],
                        start=(dk == 0), stop=(dk == NDC - 1))

            # ---- stage B: copy S to SBUF, PE-transpose to t-major,
            # min/max via lane-parallel reduce + partition_all_reduce ----
            for tcn in range(NTC):
                seg = stS[0:J, tcn * 512:(tcn + 1) * 512]
                nc.scalar.copy(seg, psA[tcn][:])
            psa_pool.__exit__(None, None, None)
            mem_sb = consts.tile([128, NDC * MEM], F32, tag="memsb")
            nc.sync.dma_start(out=mem_sb[:], in_=memsb[:])
            samp_sb = consts.tile([128, NDC * SN], F32, tag="sampsb")
            nc.sync.dma_start(out=samp_sb[:], in_=sampsb[:])
            pstp = tc.tile_pool(name="pst", bufs=1, space="PSUM")
            pst = pstp.__enter__()
            miscp = tc.tile_pool(name="misc", bufs=3, space="PSUM")
            misc = miscp.__enter__()
            psTT = pst.tile([128, NCH * J], F32, tag="pstt", name="psTT")
            for gc in range(NCH):
                nc.tensor.transpose(psTT[:, gc * J:(gc + 1) * J],
                                    stS[0:J, gc * 128:(gc + 1) * 128],
                                    ident10[0:J, 0:J])
            mmB = sb.tile([128, 2 * J], F32, tag="mmb")
            vtt = psTT[:].rearrange("p (g j) -> p j g", j=J)
            nc.vector.tensor_reduce(mmB[:, 0:J], vtt, AX.X, OP.max)
            nc.vector.tensor_reduce(mmB[:, J:2 * J], vtt, AX.X, OP.min,
                                    negate=True)
            mmA = sb.tile([128, 2 * J], F32, tag="mma")
            nc.gpsimd.partition_all_reduce(mmA[:], mmB[:], 128,
                                           bass_isa.ReduceOp.max)
            cbA = dram.tile([1, 2 * J], F32, tag="cba")
            cbB = dram.tile([1, 2 * J], F32, tag="cbb")
            nc.sync.dma_start(out=cbA[:], in_=mmA[0:1, :])
            if sim1:
                nc.gpsimd.dma_start(out=cbB[:], in_=cbA[:])
            else:
                nc.gpsimd.collective_compute("AllReduce", OP.max,
                                             replica_groups=rg,
                                             ins=[cbA.opt()],
                                             outs=[cbB.opt()])
            gmm = sb.tile([1, 2 * J], F32, tag="gmm")
            nc.sync.dma_start(out=gmm[:], in_=cbB[:])

            # scale factors (row vectors at partition 0):
            # s1 = 10/(range+1e-6); b1 = -min*s1-0.5; ds = s1*invc
            gms = sb.tile([1, 2 * J], F32, tag="gms")
            nc.vector.tensor_tensor(gms[:], gmm[:], invc2_sb[:], OP.mult)
            den = sb.tile([1, J], F32, tag="den")
            nc.vector.scalar_tensor_tensor(den[:], gms[:, 0:J], 1e-6,
                                           gms[:, J:2 * J], OP.add, OP.add)
            s1row = sb.tile([1, J], F32, tag="s1row")
            nc.vector.reciprocal(s1row[:], den[:])
            nc.vector.tensor_scalar(s1row[:], s1row[:], 10.0, None, OP.mult)
            b1row = sb.tile([1, J], F32, tag="b1row")
            nc.vector.tensor_tensor(b1row[:], gms[:, J:2 * J], s1row[:],
                                    OP.mult)
            nc.vector.tensor_scalar(b1row[:], b1row[:], -0.5, None, OP.add)
            dsrow = sb.tile([1, J], F32, tag="dsrow")
            nc.vector.tensor_tensor(dsrow[:], s1row[:], invc2_sb[:, 0:J],
                                    OP.mult)

            rmat = sb.tile([J + 1, J], F32, tag="rmat")
            dsb_ps = misc.tile([J, J], F32, tag="m", name="dsb_ps")
            nc.tensor.matmul(dsb_ps[:], ones1_10[:, 0:J], dsrow[:],
                             start=True, stop=True)
            dsb = sb.tile([J, J], F32, tag="dsb")
            nc.scalar.copy(dsb[:], dsb_ps[:])
            nc.gpsimd.affine_select(out=rmat[0:J, :], in_=dsb[:],
                                    compare_op=OP.is_equal, fill=0.0, base=0,
                                    pattern=[[-1, J]], channel_multiplier=1)
            nc.sync.dma_start(out=rmat[J:J + 1, :], in_=b1row[:])

            # ---- stage C: affine+transpose via PE, bin, one-hot ----
            psC = pst.tile([128, NCH * J], F32, tag="psc", name="psC")
            for gc in range(NCH):
                nc.tensor.matmul(
                    psC[:, gc * J:(gc + 1) * J],
                    stS[0:J + 1, gc * 128:(gc + 1) * 128],
                    rmat[:], start=True, stop=True)
            binint = sb.tile([128, NCH * J], I32, tag="binint")
            nc.vector.tensor_copy(binint[:], psC[:])
            nc.vector.tensor_scalar(binint[:], binint[:], 0, NB - 1, OP.max,
                                    OP.min)
            # one-hot, bf16: per chunk 160 cols, x-blocks 32-spaced
            # (cols 32p+b, zero-padded) so joint diag blocks land at
            # partition bases 0/32/64/96; y-pack contiguous at 112+10p+b
            CW = 160
            ohsb = sb.tile([128, NCH * CW], BF16, tag="ohsb")
            nc.gpsimd.memset(ohsb[:], 0.0)
            oh3 = ohsb[:].rearrange("pt (c r) -> pt c r", r=CW)
            ohx = oh3[:, :, 0:128].rearrange("pt c (p b2) -> pt c p b2",
                                             b2=32)[:, :, :, 0:NB]
            ohy = oh3[:, :, 112:152].rearrange("pt c (p b) -> pt c p b",
                                               b=NB)
            bi4 = binint[:].rearrange("pt (c p s) -> pt c s p", c=NCH,
                                      p=NPAIR, s=2)
            io4 = iota10[:].rearrange("pt (x y b) -> pt x y b", x=1, y=1)
            for s, dst in ((0, ohx), (1, ohy)):
                bi = bi4[:, :, s, :][:, :, :, None]
                nc.vector.tensor_tensor(
                    dst,
                    bi.broadcast_to([128, NCH, NPAIR, NB]),
                    io4.broadcast_to([128, NCH, NPAIR, NB]),
                    OP.is_equal)

            # ---- joint histograms: packed [106,40] accumulation ----
            psJ = pst.tile([128, NPAIR * NB], F32, tag="pj", name="psJ")
            for c in range(NCH):
                nc.tensor.matmul(psJ[0:106, :],
                                 ohsb[:, c * CW:c * CW + 106],
                                 ohsb[:, c * CW + 112:c * CW + 152],
                                 start=(c == 0), stop=(c == NCH - 1))
            jm = sb.tile([128, NB], F32, tag="jm")
            for p in range(NPAIR):
                nc.scalar.copy(jm[32 * p:32 * p + NB, :],
                               psJ[32 * p:32 * p + NB, NB * p:NB * (p + 1)])
            cbj = dram.tile([NPAIR, NB * NB], F32, tag="cbj")
            cbj2 = dram.tile([NPAIR, NB * NB], F32, tag="cbj2")
            qs = [nc.sync, nc.scalar, nc.sync, nc.scalar]
            for p in range(NPAIR):
                qs[p].dma_start(
                    out=cbj[p:p + 1, :].rearrange("x (a b) -> (x a) b",
                                                  b=NB),
                    in_=jm[32 * p:32 * p + NB, :])
            if sim1:
                nc.gpsimd.dma_start(out=cbj2[:], in_=cbj[:])
            else:
                nc.gpsimd.collective_compute("AllReduce", OP.add,
                                             replica_groups=rg,
                                             ins=[cbj.opt()],
                                             outs=[cbj2.opt()])
            gj = sb.tile([NPAIR * NB, NB], F32, tag="gj")
            nc.scalar.dma_start(
                out=gj[:],
                in_=cbj2[:].rearrange("p (a b) -> (p a) b", b=NB))

            outrow = sb.tile([1, 9], F32, tag="outrow")
            # ---- differentiation branch (tail; Pool/DVE/PE idle-ish) ----
            psG = misc.tile([SN, SN], F32, tag="m", name="psG")
            for k in range(NDC):
                nc.tensor.matmul(psG[:], samp_sb[:, k * SN:(k + 1) * SN],
                                 samp_sb[:, k * SN:(k + 1) * SN],
                                 start=(k == 0), stop=(k == NDC - 1))
            sqs = sb.tile([128, NDC * SN], F32, tag="sqs")
            nc.vector.tensor_tensor(sqs[:], samp_sb[:], samp_sb[:], OP.mult)
            psr = misc.tile([SN, 1], F32, tag="m", name="psr")
            for k in range(NDC):
                nc.tensor.matmul(psr[:], sqs[:, k * SN:(k + 1) * SN],
                                 ones128[:], start=(k == 0),
                                 stop=(k == NDC - 1))
            g_sb = sb.tile([SN, SN], F32, tag="gsb")
            nc.scalar.copy(g_sb[:], psG[:])
            r_sb = sb.tile([SN, 1], F32, tag="rsb")
            nc.scalar.copy(r_sb[:], psr[:])

            # variance via E[x^2]-E[x]^2 (unbiased): Pool reduces, DVE sq
            mem3 = mem_sb[:].rearrange("p (k f) -> p k f", f=MEM)
            msum = sb.tile([128, NDC], F32, tag="msum")
            nc.vector.tensor_reduce(msum[:], mem3, AX.X, OP.add)
            sq = sb.tile([128, NDC * MEM], F32, tag="sq")
            nc.vector.tensor_tensor(sq[:], mem_sb[:], mem_sb[:], OP.mult)
            s2sum = sb.tile([128, NDC], F32, tag="s2sum")
            nc.vector.tensor_reduce(
                s2sum[:], sq[:].rearrange("p (k f) -> p k f", f=MEM),
                AX.X, OP.add)
            var16 = sb.tile([128, NDC], F32, tag="var16")
            nc.vector.tensor_tensor(var16[:], msum[:], msum[:], OP.mult)
            nc.vector.tensor_scalar(var16[:], var16[:],
                                    float(-1.0 / MEM), None, OP.mult)
            nc.vector.tensor_tensor(var16[:], var16[:], s2sum[:], OP.add)
            nc.vector.tensor_scalar(var16[:], var16[:],
                                    float(1.0 / (MEM - 1)), None, OP.mult)
            redv = sb.tile([128, 1], F32, tag="redv")
            nc.vector.tensor_reduce(redv[:], var16[:], AX.X, OP.add)
            v2 = sb.tile([128, NDC], F32, tag="v2")
            nc.vector.tensor_tensor(v2[:], var16[:], var16[:], OP.mult)
            redv2 = sb.tile([128, 1], F32, tag="redv2")
            nc.vector.tensor_reduce(redv2[:], v2[:], AX.X, OP.add)
            pstv = misc.tile([1, 1], F32, tag="m", name="pstv")
            nc.tensor.matmul(pstv[:], redv[:], ones128[:], start=True,
                             stop=True)
            tv_sb = sb.tile([1, 1], F32, tag="tvsb")
            nc.scalar.copy(tv_sb[:], pstv[:])
            pss2 = misc.tile([1, 1], F32, tag="m", name="pss2")
            nc.tensor.matmul(pss2[:], redv2[:], ones128[:], start=True,
                             stop=True)
            s2_sb = sb.tile([1, 1], F32, tag="s2sb")
            nc.scalar.copy(s2_sb[:], pss2[:])

            tvsq = sb.tile([1, 1], F32, tag="tvsq")
            nc.vector.tensor_tensor(tvsq[:], tv_sb[:], tv_sb[:], OP.mult)
            dden = sb.tile([1, 1], F32, tag="dden")
            nc.vector.scalar_tensor_tensor(dden[:], tvsq[:], 1e-6, s2_sb[:],
                                           OP.mult, OP.add)
            rdden = sb.tile([1, 1], F32, tag="rdden")
            nc.vector.reciprocal(rdden[:], dden[:])
            nc.vector.tensor_tensor(outrow[:, 2:3], tvsq[:], rdden[:],
                                    OP.mult)

            # cdist tail: d2 = r_i + r_j - 2G
            rrow_ps = misc.tile([1, SN], F32, tag="m", name="rrow_ps")
            nc.tensor.transpose(rrow_ps[:], r_sb[:], ident10[:])
            rrow = sb.tile([1, SN], F32, tag="rrow")
            nc.scalar.copy(rrow[:], rrow_ps[:])
            rB = misc.tile([SN, SN], F32, tag="m", name="rB")
            nc.tensor.matmul(rB[:], ones1_10[:], rrow[:], start=True,
                             stop=True)
            d2 = sb.tile([SN, SN], F32, tag="d2")
            nc.vector.scalar_tensor_tensor(d2[:], g_sb[:], -2.0, rB[:],
                                           OP.mult, OP.add)
            nc.vector.tensor_scalar(d2[:], d2[:], r_sb[:], 1e-20, OP.add,
                                    OP.max)
            dst = sb.tile([SN, SN], F32, tag="dst")
            nc.scalar.activation(dst[:], d2[:], ACT.Sqrt)
            dsum = sb.tile([SN, 1], F32, tag="dsum")
            nc.vector.tensor_reduce(dsum[:], dst[:], AX.X, OP.add)
            psD = misc.tile([1, 1], F32, tag="m", name="psD")
            nc.tensor.matmul(psD[:], dsum[:], ones10[:], start=True,
                             stop=True)
            avg_sb = sb.tile([1, 1], F32, tag="avgsb")
            nc.vector.tensor_scalar(avg_sb[:], psD[:],
                                    float(1.0 / (SN * (SN - 1) + 1e-6)),
                                    None, OP.mult)
            sqtv = sb.tile([1, 1], F32, tag="sqtv")
            nc.scalar.activation(sqtv[:], tv_sb[:], ACT.Sqrt)
            nc.vector.tensor_tensor(outrow[:, 1:2], sqtv[:], avg_sb[:],
                                    OP.mult)
            tanhd = sb.tile([1, 1], F32, tag="tanhd")
            nc.scalar.activation(tanhd[:], outrow[:, 1:2], ACT.Tanh)
            nc.vector.tensor_copy(outrow[:, 3:4], tv_sb[:])
            # ---- stage D: MI for all 4 pairs at once (40 lanes) ----
            rowsum = sb.tile([NPAIR * NB, 1], F32, tag="rowsum")
            nc.vector.tensor_reduce(rowsum[:], gj[:], AX.X, OP.add)
            colps = misc.tile([NPAIR, NB], F32, tag="m", name="colps")
            nc.tensor.matmul(colps[:], selc_sb[:], gj[:], start=True,
                             stop=True)
            py4 = sb.tile([NPAIR, NB], F32, tag="py4")
            nc.vector.tensor_scalar(py4[:], colps[:], TINV, None, OP.mult)
            jn = sb.tile([NPAIR * NB, NB], F32, tag="jn")
            nc.vector.tensor_scalar(jn[:], gj[:], TINV, None, OP.mult)
            px = sb.tile([NPAIR * NB, 1], F32, tag="px")
            nc.vector.tensor_scalar(px[:], rowsum[:], TINV, None, OP.mult)
            pyB = misc.tile([NPAIR * NB, NB], F32, tag="m", name="pyB")
            nc.tensor.matmul(pyB[:], selcT_sb[:], py4[:], start=True,
                             stop=True)
            outer = sb.tile([NPAIR * NB, NB], F32, tag="outer")
            nc.vector.tensor_scalar(outer[:], pyB[:], px[:], 1e-10,
                                    OP.mult, OP.add)
            num = sb.tile([NPAIR * NB, NB], F32, tag="num")
            nc.vector.tensor_scalar(num[:], jn[:], 1e-10, None, OP.add)
            rout = sb.tile([NPAIR * NB, NB], F32, tag="rout")
            nc.vector.reciprocal(rout[:], outer[:])
            nc.vector.tensor_tensor(num[:], num[:], rout[:], OP.mult)
            lg = sb.tile([NPAIR * NB, NB], F32, tag="lg")
            nc.scalar.activation(lg[:], num[:], ACT.Ln)
            nc.vector.tensor_tensor(lg[:], jn[:], lg[:], OP.mult)
            ms = sb.tile([NPAIR * NB, 1], F32, tag="ms")
            nc.vector.tensor_reduce(ms[:], lg[:], AX.X, OP.add)
            mi4 = misc.tile([NPAIR, 1], F32, tag="m", name="mi4")
            nc.tensor.matmul(mi4[:], selc_sb[:], ms[:], start=True,
                             stop=True)
            mi4sb = sb.tile([NPAIR, 1], F32, tag="mi4sb")
            nc.scalar.copy(mi4sb[:], mi4[:])
            mirps = misc.tile([1, NPAIR], F32, tag="m", name="mirps")
            nc.tensor.transpose(mirps[:], mi4sb[:],
                                ident10[0:NPAIR, 0:NPAIR])
            nc.vector.tensor_scalar(outrow[:, 5:9], mirps[:], 0.0, None,
                                    OP.max)
            nc.vector.tensor_reduce(outrow[:, 4:5], outrow[:, 5:9], AX.X,
                                    OP.min)

            nc.vector.tensor_tensor(outrow[:, 0:1], outrow[:, 4:5],
                                    tanhd[:], OP.add)
            nc.sync.dma_start(out=out[:], in_=outrow[:])
            miscp.__exit__(None, None, None)
            pstp.__exit__(None, None, None)

    nc.compile()
    return nc


def _get_nc(debug=False):
    key = ("ncd" if debug else "nc")
    if key not in _CACHE:
        _CACHE[key] = _build(debug)
    return _CACHE[key]


def kernel(state, state_memory, state_history, partitions, sample_idx,
           trace=False, debug=False):
    global LAST_RESULTS
    state = np.asarray(state, np.float32)
    state_memory = np.asarray(state_memory, np.float32)
    state_history = np.asarray(state_history, np.float32)
    partitions = np.asarray(partitions)
    sample_idx = np.asarray(sample_idx)

    mmat = np.empty((D, J), np.float32)
    invc8 = np.empty((J,), np.float32)
    pf = partitions.astype(np.float32)
    for p in range(NPAIR):
        mmat[:, 2 * p] = pf[p]
        mmat[:, 2 * p + 1] = np.float32(1.0) - pf[p]
        invc8[2 * p] = np.float32(1.0) / pf[p].sum(dtype=np.float32)
        invc8[2 * p + 1] = np.float32(1.0) / (np.float32(1.0)
                                              - pf[p]).sum(dtype=np.float32)
    invc2 = np.concatenate([invc8, invc8]).reshape(1, 2 * J)
    # SBUF layouts precomputed host-side: [128, k*cols] with row d = k*128+p
    msb = np.ascontiguousarray(
        mmat.reshape(NDC, 128, J).transpose(1, 0, 2).reshape(128, NDC * J))
    memory = np.concatenate([state, state_memory[state.shape[0]:]], axis=0)
    memsb = np.ascontiguousarray(
        memory.T.reshape(NDC, 128, MEM).transpose(1, 0, 2).reshape(
            128, NDC * MEM))
    sampsb = np.ascontiguousarray(
        memory[sample_idx].T.reshape(NDC, 128, SN).transpose(1, 0, 2).reshape(
            128, NDC * SN))
    selcm = np.zeros((NPAIR * NB, NPAIR), np.float32)
    for p in range(NPAIR):
        selcm[NB * p:NB * (p + 1), p] = 1.0
    selcT = np.ascontiguousarray(selcm.T)

    in_maps = []
    for c in range(N_CORES):
        htc = np.ascontiguousarray(state_history[c * TL:(c + 1) * TL, :].T)
        in_maps.append({"ht": htc, "msb": msb, "invc2": invc2,
                        "memsb": memsb, "sampsb": sampsb,
                        "selc": selcm, "selcT": selcT})

    nc = _get_nc(debug)
    res = run_bass_kernel_spmd(nc, in_maps, list(range(N_CORES)),
                               trace=trace)
    LAST_RESULTS = res
    return np.asarray(res.results[0]["out"], np.float32)



# revision 29
# speedup vs baseline: 1.0807x; 1.0807x over previous
"""Trainium2 Bass kernel for nn_ConsciousnessMonitor (histogram_binning).

kernel(**inputs) takes FULL unsharded numpy inputs, returns the full (9,)
float32 output. Shards state_history along time across 8 NeuronCores.

Stage A streams HT in d-major [128, TL] chunks (DMA-bound, ~360 GB/s) and
computes S = HT.T @ M directly t-major into PSUM via 512 small fp32r
matmuls (ht chunk stationary, mask matrix moving), so no PSUM->SBUF copies
or PE transposes are needed before the min/max reduction. The last d-chunk
is split into 4 quarter-DMAs so the post-stream PE tail is tiny. Min/max
reduce straight off PSUM + partition_all_reduce -> AllReduce(max). The
affine bin transform broadcasts ds/b1 across partitions (gpsimd
partition_broadcast) and runs on DVE; dense bf16 one-hots (80 cols/chunk,
no memset) feed one packed [40,40] PSUM histogram accumulation -> single
[40,10] DMA -> AllReduce(add). MI tail works in count space with 1/T
folded into host-side selector constants. The differentiation branch is
gated behind collective-1's output DMA (runs inside the collective
window); its variance sums use Activation-engine accumulate so DVE stays
free for the binning critical path; tanh is computed via exp so Ln/Exp
share one activation table set.

Self-contained: shapes/sharding hardcoded; reads no sibling files.
"""
import numpy as np

import concourse.bacc as bacc
import concourse.tile as tile
import concourse.mybir as mybir
from concourse.bass_utils import run_bass_kernel_spmd
import concourse.bass_isa as bass_isa

F32 = mybir.dt.float32
F32R = mybir.dt.float32r
BF16 = mybir.dt.bfloat16
I32 = mybir.dt.int32
AX = mybir.AxisListType
OP = mybir.AluOpType
ACT = mybir.ActivationFunctionType

N_CORES = 8
T, D = 32768, 2048
TL = T // N_CORES          # 4096 time steps per core
NB = 10                    # histogram bins per axis
NPAIR = 4                  # partitions (mask pairs)
J = 2 * NPAIR              # 8 masked-mean columns
NDC = D // 128             # 16 contraction chunks
NCH = TL // 128            # 32 t-chunks of 128
CW = 2 * NPAIR * NB        # 80 one-hot cols per chunk (x pack + y pack)
MEM = 100
SN = 10
TINV = float(1.0 / (np.float32(T) + np.float32(1e-10)))

_CACHE = {}
LAST_RESULTS = None


def _build(debug=False, variant="main"):
    return _build_inner(debug, variant)


def _build_inner(debug, variant):
    sim1 = variant.startswith("sim1")
    nc = bacc.Bacc("TRN2", target_bir_lowering=False, debug=False,
                   num_devices=1 if sim1 else N_CORES)
    ht = nc.dram_tensor("ht", [D, TL], F32, kind="ExternalInput").ap()
    msb = nc.dram_tensor("msb", [128, NDC * J], F32,
                         kind="ExternalInput").ap()
    invc2 = nc.dram_tensor("invc2", [1, 2 * J], F32,
                           kind="ExternalInput").ap()
    memsb = nc.dram_tensor("memsb", [128, NDC * MEM], F32,
                           kind="ExternalInput").ap()
    sampsb = nc.dram_tensor("sampsb", [128, NDC * SN], F32,
                            kind="ExternalInput").ap()
    selB = nc.dram_tensor("selB", [NPAIR * NB, NPAIR * NB], F32,
                          kind="ExternalInput").ap()
    maskc = nc.dram_tensor("maskc", [NPAIR * NB, NPAIR * NB], F32,
                           kind="ExternalInput").ap()
    selcR = nc.dram_tensor("selcR", [NPAIR * NB, NPAIR], F32,
                           kind="ExternalInput").ap()
    out = nc.dram_tensor("out", [9], F32, kind="ExternalOutput").ap()
    dbg = variant == "dbg"
    if dbg:
        dbg_gmm = nc.dram_tensor("dbg_gmm", [1, 2 * J], F32,
                                 kind="ExternalOutput").ap()
        dbg_row = nc.dram_tensor("dbg_row", [1, 2 * J], F32,
                                 kind="ExternalOutput").ap()
        dbg_bin = nc.dram_tensor("dbg_bin", [128, NCH * J], I32,
                                 kind="ExternalOutput").ap()
        dbg_bin2 = nc.dram_tensor("dbg_bin2", [128, NCH * J], F32,
                                  kind="ExternalOutput").ap()
        dbg_gj = nc.dram_tensor("dbg_gj", [NPAIR * NB, NPAIR * NB], F32,
                                kind="ExternalOutput").ap()

    rg = [list(range(N_CORES))]

    with tile.TileContext(nc) as tc:
        with tc.tile_pool(name="consts", bufs=1) as consts, \
             tc.tile_pool(name="sb", bufs=1) as sb, \
             tc.tile_pool(name="htp", bufs=3) as htp, \
             tc.tile_pool(name="pst", bufs=1, space="PSUM") as pst, \
             tc.tile_pool(name="misc", bufs=3, space="PSUM") as misc, \
             tc.tile_pool(name="dram", bufs=1, space="DRAM") as dram:

            # ---- constants on the gpsimd queue so the ht stream owns the
            # sync queue from t=0 ----
            m_sb = consts.tile([128, NDC * J], F32, tag="msb")
            nc.gpsimd.dma_start(out=m_sb[:], in_=msb[:])
            invc2_sb = consts.tile([1, 2 * J], F32, tag="invc2")
            nc.gpsimd.dma_start(out=invc2_sb[:], in_=invc2[:])
            selB_sb = consts.tile([NPAIR * NB, NPAIR * NB], F32, tag="selb")
            nc.gpsimd.dma_start(out=selB_sb[:], in_=selB[:])
            maskc_sb = consts.tile([NPAIR * NB, NPAIR * NB], F32,
                                   tag="maskc")
            nc.gpsimd.dma_start(out=maskc_sb[:], in_=maskc[:])
            selcR_sb = consts.tile([NPAIR * NB, NPAIR], F32, tag="selcr")
            nc.gpsimd.dma_start(out=selcR_sb[:], in_=selcR[:])

            ones128 = consts.tile([128, 1], F32, tag="o128")
            nc.gpsimd.memset(ones128[:], 1.0)
            ones1_10 = consts.tile([1, NB], F32, tag="o110")
            nc.gpsimd.memset(ones1_10[:], 1.0)
            ones10 = consts.tile([NB, 1], F32, tag="o10")
            nc.gpsimd.memset(ones10[:], 1.0)
            iota10 = consts.tile([128, NB], I32, tag="iota10")
            nc.gpsimd.iota(iota10[:], pattern=[[1, NB]], base=0,
                           channel_multiplier=0)

            # ---- stage A: stream HT, S = HT.T @ M accumulated t-major in
            # PSUM ([128 t, NCH*J]); last d-chunk split into quarters so PE
            # finishes almost with the stream ----
            # a matmul with start=True resets the WHOLE PSUM bank on real
            # HW, so 32 interleaved accumulation groups in one bank would
            # corrupt each other: zero the bank once and accumulate with
            # start=False everywhere instead
            psS = pst.tile([128, NCH * J], F32, tag="pss", name="psS")
            nc.vector.memset(psS[:], 0.0)
            for dk in range(NDC - 1):
                htt = htp.tile([128, TL], F32, tag="htt", name="htt")
                nc.sync.dma_start(out=htt[:],
                                  in_=ht[dk * 128:(dk + 1) * 128, :])
                for tcn in range(NCH):
                    nc.tensor.matmul(
                        psS[:, tcn * J:(tcn + 1) * J],
                        htt[:, tcn * 128:(tcn + 1) * 128],
                        m_sb[:, dk * J:(dk + 1) * J],
                        start=False, stop=False, skip_group_check=True)
            dk = NDC - 1
            QT = TL // 4
            for q in range(4):
                htq = sb.tile([128, QT], F32, tag=f"htq{q}")
                dmaq = nc.sync if q % 2 == 0 else nc.scalar
                dmaq.dma_start(
                    out=htq[:],
                    in_=ht[dk * 128:(dk + 1) * 128, q * QT:(q + 1) * QT])
                for tq in range(QT // 128):
                    tcn = q * (QT // 128) + tq
                    nc.tensor.matmul(
                        psS[:, tcn * J:(tcn + 1) * J],
                        htq[:, tq * 128:(tq + 1) * 128],
                        m_sb[:, dk * J:(dk + 1) * J],
                        start=False, stop=True, skip_group_check=True)

            # ---- min/max straight off PSUM, then cross-core AllReduce ----
            mmB = sb.tile([128, 2 * J], F32, tag="mmb")
            vt = psS[:].rearrange("p (c j) -> p j c", j=J)
            nc.vector.tensor_reduce(mmB[:, 0:J], vt, AX.X, OP.max)
            nc.vector.tensor_reduce(mmB[:, J:2 * J], vt, AX.X, OP.min,
                                    negate=True)
            mmA = sb.tile([128, 2 * J], F32, tag="mma")
            nc.gpsimd.partition_all_reduce(mmA[:], mmB[:], 128,
                                           bass_isa.ReduceOp.max)
            cbA = dram.tile([1, 2 * J], F32, tag="cba")
            cbB = dram.tile([1, 2 * J], F32, tag="cbb")
            nc.sync.dma_start(out=cbA[:], in_=mmA[0:1, :])
            if sim1:
                nc.gpsimd.dma_start(out=cbB[:], in_=cbA[:])
            else:
                nc.gpsimd.collective_compute("AllReduce", OP.max,
                                             replica_groups=rg,
                                             ins=[cbA.opt()],
                                             outs=[cbB.opt()])
            # diff-branch inputs queued on sync AFTER cbA: they land (and
            # the branch runs) inside the collective window
            mem_sb = consts.tile([128, NDC * MEM], F32, tag="memsb")
            nc.sync.dma_start(out=mem_sb[:], in_=memsb[:])
            samp_sb = consts.tile([128, NDC * SN], F32, tag="sampsb")
            nc.sync.dma_start(out=samp_sb[:], in_=sampsb[:])
            gmm = sb.tile([1, 2 * J], F32, tag="gmm")
            nc.sync.dma_start(out=gmm[:], in_=cbB[:])

            # ---- differentiation branch part 1 (hidden in collective-1
            # window: DVE is idle 98.6-104.9, Act idle until the jm copies;
            # msum goes through Activation accumulate so the DVE work fits
            # the window) ----
            psG = misc.tile([SN, SN], F32, tag="m", name="psG")
            for k in range(NDC):
                nc.tensor.matmul(psG[:], samp_sb[:, k * SN:(k + 1) * SN],
                                 samp_sb[:, k * SN:(k + 1) * SN],
                                 start=(k == 0), stop=(k == NDC - 1))
            sqs = sb.tile([128, NDC * SN], F32, tag="sqs")
            nc.vector.tensor_tensor(sqs[:], samp_sb[:], samp_sb[:], OP.mult)
            psrc = misc.tile([SN, 1], F32, tag="m", name="psrc")
            psrr = misc.tile([1, SN], F32, tag="m", name="psrr")
            for k in range(NDC):
                nc.tensor.matmul(psrc[:], sqs[:, k * SN:(k + 1) * SN],
                                 ones128[:], start=(k == 0),
                                 stop=(k == NDC - 1))
            for k in range(NDC):
                nc.tensor.matmul(psrr[:], ones128[:],
                                 sqs[:, k * SN:(k + 1) * SN],
                                 start=(k == 0), stop=(k == NDC - 1))
            g_sb = sb.tile([SN, SN], F32, tag="gsb")
            nc.scalar.copy(g_sb[:], psG[:])
            rcol = sb.tile([SN, 1], F32, tag="rcol")
            nc.scalar.copy(rcol[:], psrc[:])
            rrow = sb.tile([1, SN], F32, tag="rrow")
            nc.scalar.copy(rrow[:], psrr[:])
            rB = misc.tile([SN, SN], F32, tag="m", name="rB")
            nc.tensor.matmul(rB[:], ones1_10[:], rrow[:], start=True,
                             stop=True)
            d2 = sb.tile([SN, SN], F32, tag="d2")
            nc.vector.scalar_tensor_tensor(d2[:], g_sb[:], -2.0, rB[:],
                                           OP.mult, OP.add)
            nc.vector.tensor_scalar(d2[:], d2[:], rcol[:], 1e-20, OP.add,
                                    OP.max)
            dst = sb.tile([SN, SN], F32, tag="dst")
            nc.scalar.activation(dst[:], d2[:], ACT.Sqrt)
            dsum = sb.tile([SN, 1], F32, tag="dsum")
            nc.vector.tensor_reduce(dsum[:], dst[:], AX.X, OP.add)
            psD = misc.tile([1, 1], F32, tag="m", name="psD")
            nc.tensor.matmul(psD[:], dsum[:], ones10[:], start=True,
                             stop=True)
            avg_sb = sb.tile([1, 1], F32, tag="avgsb")
            nc.vector.tensor_scalar(avg_sb[:], psD[:],
                                    float(1.0 / (SN * (SN - 1) + 1e-6)),
                                    None, OP.mult)

            msum = sb.tile([128, NDC], F32, tag="msum")
            s2sum = sb.tile([128, NDC], F32, tag="s2sum")
            sqm = sb.tile([128, NDC * MEM], F32, tag="sqm")
            nc.vector.tensor_reduce(
                msum[:], mem_sb[:].rearrange("p (k f) -> p k f", f=MEM),
                AX.X, OP.add)
            nc.vector.tensor_tensor(sqm[:], mem_sb[:], mem_sb[:], OP.mult)
            nc.vector.tensor_reduce(
                s2sum[:], sqm[:].rearrange("p (k f) -> p k f", f=MEM),
                AX.X, OP.add)
            # variance combine + tv/sqtv/exp/ln-preload all complete inside
            # the collective-1 window so the Act table loads never touch the
            # histogram path
            var16 = sb.tile([128, NDC], F32, tag="var16")
            nc.vector.tensor_tensor(var16[:], msum[:], msum[:], OP.mult)
            nc.vector.tensor_scalar(var16[:], var16[:],
                                    float(-1.0 / MEM), None, OP.mult)
            nc.vector.tensor_tensor(var16[:], var16[:], s2sum[:], OP.add)
            nc.vector.tensor_scalar(var16[:], var16[:],
                                    float(1.0 / (MEM - 1)), None, OP.mult)
            redv = sb.tile([128, 1], F32, tag="redv")
            nc.vector.tensor_reduce(redv[:], var16[:], AX.X, OP.add)
            v2 = sb.tile([128, NDC], F32, tag="v2")
            nc.vector.tensor_tensor(v2[:], var16[:], var16[:], OP.mult)
            redv2 = sb.tile([128, 1], F32, tag="redv2")
            nc.vector.tensor_reduce(redv2[:], v2[:], AX.X, OP.add)
            pstv = misc.tile([1, 1], F32, tag="m", name="pstv")
            nc.tensor.matmul(pstv[:], redv[:], ones128[:], start=True,
                             stop=True)
            pss2 = misc.tile([1, 1], F32, tag="m", name="pss2")
            nc.tensor.matmul(pss2[:], redv2[:], ones128[:], start=True,
                             stop=True)
            outrow = sb.tile([1, 9], F32, tag="outrow")
            tv_sb = sb.tile([1, 1], F32, tag="tvsb")
            nc.vector.tensor_copy(tv_sb[:], pstv[:])
            tvsq = sb.tile([1, 1], F32, tag="tvsq")
            nc.vector.tensor_tensor(tvsq[:], tv_sb[:], tv_sb[:], OP.mult)
            dden = sb.tile([1, 1], F32, tag="dden")
            nc.vector.scalar_tensor_tensor(dden[:], tvsq[:], 1e-6, pss2[:],
                                           OP.mult, OP.add)
            rdden = sb.tile([1, 1], F32, tag="rdden")
            nc.vector.reciprocal(rdden[:], dden[:])
            nc.vector.tensor_tensor(outrow[:, 2:3], tvsq[:], rdden[:],
                                    OP.mult)
            nc.vector.tensor_copy(outrow[:, 3:4], tv_sb[:])
            sqtv = sb.tile([1, 1], F32, tag="sqtv")
            nc.scalar.activation(sqtv[:], pstv[:], ACT.Sqrt)
            nc.vector.tensor_tensor(outrow[:, 1:2], sqtv[:], avg_sb[:],
                                    OP.mult)
            # tanh(x) = (1 - e^{-2x}) / (1 + e^{-2x}); x >= 0 here so the
            # exp underflows (never overflows)
            etan = sb.tile([1, 1], F32, tag="etan")
            nc.scalar.activation(etan[:], outrow[:, 1:2], ACT.Exp,
                                 scale=-2.0)
            tb = sb.tile([1, 1], F32, tag="tb")
            nc.vector.tensor_scalar(tb[:], etan[:], 1.0, None, OP.add)
            # prime the Ln table now (tb >= 1, value unused) so the MI Ln
            # needs no load later; consuming tb forces sqrt->exp->ln order
            # on the Act queue (no redundant set reloads)
            lnpre = sb.tile([1, 1], F32, tag="lnpre")
            nc.scalar.activation(lnpre[:], tb[:], ACT.Ln)

            # ---- post-collective-1: bin scale factors ----
            gms = sb.tile([1, 2 * J], F32, tag="gms")
            nc.vector.tensor_tensor(gms[:], gmm[:], invc2_sb[:], OP.mult)
            den = sb.tile([1, J], F32, tag="den")
            nc.vector.scalar_tensor_tensor(den[:], gms[:, 0:J], 1e-6,
                                           gms[:, J:2 * J], OP.add, OP.add)
            rr = sb.tile([1, J], F32, tag="rr")
            nc.vector.reciprocal(rr[:], den[:])
            dsb1row = sb.tile([1, 2 * J], F32, tag="dsb1row")
            nc.vector.scalar_tensor_tensor(dsb1row[:, 0:J], rr[:], 10.0,
                                           invc2_sb[:, 0:J], OP.mult,
                                           OP.mult)
            nc.vector.scalar_tensor_tensor(dsb1row[:, J:2 * J], rr[:], 10.0,
                                           gms[:, J:2 * J], OP.mult,
                                           OP.mult)
            dsb1 = sb.tile([128, 2 * J], F32, tag="dsb1")
            nc.gpsimd.partition_broadcast(dsb1[:], dsb1row[:])

            # ---- bin values + one-hot + packed joint histogram ----
            ps3 = psS[:].rearrange("p (c j) -> p c j", j=J)
            dsv = dsb1[:, 0:J].rearrange("p (c j) -> p c j", c=1)
            b1v = dsb1[:, J:2 * J].rearrange("p (c j) -> p c j", c=1)
            bin1 = sb.tile([128, NCH * J], F32, tag="bin1")
            bin13 = bin1[:].rearrange("p (c j) -> p c j", j=J)
            nc.vector.tensor_tensor(bin13, ps3,
                                    dsv.broadcast_to([128, NCH, J]), OP.mult)
            bin2 = sb.tile([128, NCH * J], F32, tag="bin2")
            bin23 = bin2[:].rearrange("p (c j) -> p c j", j=J)
            nc.vector.scalar_tensor_tensor(bin23, bin13, -0.5,
                                           b1v.broadcast_to([128, NCH, J]),
                                           OP.add, OP.add)
            binint = sb.tile([128, NCH * J], I32, tag="binint")
            nc.vector.tensor_copy(binint[:], bin2[:])
            nc.vector.tensor_scalar(binint[:], binint[:], 0, NB - 1, OP.max,
                                    OP.min)
            # dense one-hot, bf16: per chunk 80 cols: x pack [p*10+b] then
            # y pack [40 + p*10+b]; fully written so no memset needed
            ohsb = sb.tile([128, NCH * CW], BF16, tag="ohsb")
            oh3 = ohsb[:].rearrange("pt (c r) -> pt c r", r=CW)
            ohx = oh3[:, :, 0:NPAIR * NB].rearrange(
                "pt c (p b) -> pt c p b", b=NB)
            ohy = oh3[:, :, NPAIR * NB:2 * NPAIR * NB].rearrange(
                "pt c (p b) -> pt c p b", b=NB)
            bi4 = binint[:].rearrange("pt (c p s) -> pt c s p", c=NCH,
                                      p=NPAIR, s=2)
            io4 = iota10[:].rearrange("pt (x y b) -> pt x y b", x=1, y=1)
            psJ = misc.tile([NPAIR * NB, NPAIR * NB], F32, tag="m",
                            name="psJ")
            HALF = NCH // 2
            for h in range(2):
                c0, c1 = h * HALF, (h + 1) * HALF
                for s, dstv in ((0, ohx), (1, ohy)):
                    bi = bi4[:, c0:c1, s, :][:, :, :, None]
                    nc.vector.tensor_tensor(
                        dstv[:, c0:c1],
                        bi.broadcast_to([128, HALF, NPAIR, NB]),
                        io4.broadcast_to([128, HALF, NPAIR, NB]),
                        OP.is_equal)
                for c in range(c0, c1):
                    nc.tensor.matmul(
                        psJ[:],
                        ohsb[:, c * CW:c * CW + NPAIR * NB],
                        ohsb[:, c * CW + NPAIR * NB:(c + 1) * CW],
                        start=(c == 0), stop=(c == NCH - 1))
            # ship the whole [40,40] joint; cross-block cells are handled
            # exactly in the MI tail (full rowsum = 4R, blockdiag colsum = C)
            jm = sb.tile([NPAIR * NB, NPAIR * NB], F32, tag="jm")
            nc.scalar.copy(jm[:], psJ[:])
            cbj = dram.tile([NPAIR * NB, NPAIR * NB], F32, tag="cbj")
            cbj2 = dram.tile([NPAIR * NB, NPAIR * NB], F32, tag="cbj2")
            nc.sync.dma_start(out=cbj[:], in_=jm[:])
            if sim1:
                nc.gpsimd.dma_start(out=cbj2[:], in_=cbj[:])
            else:
                nc.gpsimd.collective_compute("AllReduce", OP.add,
                                             replica_groups=rg,
                                             ins=[cbj.opt()],
                                             outs=[cbj2.opt()])

            # ---- tanh combine (tiny DVE ops, hidden in collective-2) ----
            ta = sb.tile([1, 1], F32, tag="ta")
            nc.vector.tensor_scalar(ta[:], etan[:], -1.0, 1.0, OP.mult,
                                    OP.add)
            trb = sb.tile([1, 1], F32, tag="trb")
            nc.vector.reciprocal(trb[:], tb[:])
            tanhd = sb.tile([1, 1], F32, tag="tanhd")
            nc.vector.tensor_tensor(tanhd[:], ta[:], trb[:], OP.mult)

            gj = sb.tile([NPAIR * NB, NPAIR * NB], F32, tag="gj")
            nc.scalar.dma_start(out=gj[:], in_=cbj2[:])

            # ---- MI tail in count space on the full [40,40] joint.
            # Full rowsum = 4R (each y-block contributes R once; /4 folded
            # into selB); selB blockdiag colsum gives C*TINV^2/4 for every
            # column; cross-block cells get finite garbage lg that is
            # zeroed by the blockdiag mask before the weighted sum. ----
            rowsum = sb.tile([NPAIR * NB, 1], F32, tag="rowsum")
            nc.vector.tensor_reduce(rowsum[:], gj[:], AX.X, OP.add)
            gjm = sb.tile([NPAIR * NB, NPAIR * NB], F32, tag="gjm")
            nc.vector.tensor_tensor(gjm[:], gj[:], maskc_sb[:], OP.mult)
            psOut = misc.tile([NPAIR * NB, NPAIR * NB], F32, tag="m",
                              name="psOut")
            nc.tensor.matmul(psOut[:], selB_sb[:], gj[:], start=True,
                             stop=True)
            outer = sb.tile([NPAIR * NB, NPAIR * NB], F32, tag="outer")
            nc.vector.tensor_scalar(outer[:], psOut[:], rowsum[:], 1e-10,
                                    OP.mult, OP.add)
            rout = sb.tile([NPAIR * NB, NPAIR * NB], F32, tag="rout")
            nc.vector.reciprocal(rout[:], outer[:])
            num = sb.tile([NPAIR * NB, NPAIR * NB], F32, tag="num")
            nc.vector.tensor_scalar(num[:], gj[:], TINV, 1e-10, OP.mult,
                                    OP.add)
            nc.vector.tensor_tensor(num[:], num[:], rout[:], OP.mult)
            lg = sb.tile([NPAIR * NB, NPAIR * NB], F32, tag="lg")
            nc.scalar.activation(lg[:], num[:], ACT.Ln)
            nc.vector.tensor_tensor(lg[:], gjm[:], lg[:], OP.mult)
            ms = sb.tile([NPAIR * NB, 1], F32, tag="ms")
            nc.vector.tensor_reduce(ms[:], lg[:], AX.X, OP.add)
            psRow = misc.tile([1, NPAIR], F32, tag="m", name="psRow")
            nc.tensor.matmul(psRow[:], ms[:], selcR_sb[:], start=True,
                             stop=True)
            nc.vector.tensor_scalar(outrow[:, 5:9], psRow[:], 0.0, None,
                                    OP.max)
            nc.vector.tensor_reduce(outrow[:, 4:5], outrow[:, 5:9], AX.X,
                                    OP.min)
            nc.vector.tensor_tensor(outrow[:, 0:1], outrow[:, 4:5],
                                    tanhd[:], OP.add)
            nc.sync.dma_start(out=out[:], in_=outrow[:])
            if dbg:
                nc.sync.dma_start(out=dbg_gmm[:], in_=gmm[:])
                nc.sync.dma_start(out=dbg_row[:], in_=dsb1row[:])
                nc.sync.dma_start(out=dbg_bin[:], in_=binint[:])
                nc.sync.dma_start(out=dbg_bin2[:], in_=bin2[:])
                nc.sync.dma_start(out=dbg_gj[:], in_=gj[:])

    nc.compile()
    return nc


def _get_nc(debug=False):
    key = ("ncd" if debug else "nc")
    if key not in _CACHE:
        _CACHE[key] = _build(debug)
    return _CACHE[key]


def kernel(state, state_memory, state_history, partitions, sample_idx,
           trace=False, debug=False):
    global LAST_RESULTS
    state = np.asarray(state, np.float32)
    state_memory = np.asarray(state_memory, np.float32)
    state_history = np.asarray(state_history, np.float32)
    partitions = np.asarray(partitions)
    sample_idx = np.asarray(sample_idx)

    mmat = np.empty((D, J), np.float32)
    invc8 = np.empty((J,), np.float32)
    pf = partitions.astype(np.float32)
    for p in range(NPAIR):
        mmat[:, 2 * p] = pf[p]
        mmat[:, 2 * p + 1] = np.float32(1.0) - pf[p]
        invc8[2 * p] = np.float32(1.0) / pf[p].sum(dtype=np.float32)
        invc8[2 * p + 1] = np.float32(1.0) / (np.float32(1.0)
                                              - pf[p]).sum(dtype=np.float32)
    invc2 = np.concatenate([invc8, invc8]).reshape(1, 2 * J)
    # SBUF layouts precomputed host-side: [128, k*cols] with row d = k*128+p
    msb = np.ascontiguousarray(
        mmat.reshape(NDC, 128, J).transpose(1, 0, 2).reshape(128, NDC * J))
    memory = np.concatenate([state, state_memory[state.shape[0]:]], axis=0)
    memsb = np.ascontiguousarray(
        memory.T.reshape(NDC, 128, MEM).transpose(1, 0, 2).reshape(
            128, NDC * MEM))
    sampsb = np.ascontiguousarray(
        memory[sample_idx].T.reshape(NDC, 128, SN).transpose(1, 0, 2).reshape(
            128, NDC * SN))
    selcm = np.zeros((NPAIR * NB, NPAIR), np.float32)
    for p in range(NPAIR):
        selcm[NB * p:NB * (p + 1), p] = 1.0
    selcR = np.ascontiguousarray(selcm * np.float32(TINV))
    # block-diagonal [40,40] constants: selB = TINV^2/4 (full rowsum = 4R),
    # maskc = 1.0 on the diagonal blocks
    maskc = np.kron(np.eye(NPAIR, dtype=np.float32),
                    np.ones((NB, NB), np.float32))
    selB = np.ascontiguousarray(maskc * np.float32(TINV * TINV / 4.0))
    maskc = np.ascontiguousarray(maskc)

    in_maps = []
    for c in range(N_CORES):
        htc = np.ascontiguousarray(state_history[c * TL:(c + 1) * TL, :].T)
        in_maps.append({"ht": htc, "msb": msb, "invc2": invc2,
                        "memsb": memsb, "sampsb": sampsb,
                        "selB": selB, "maskc": maskc, "selcR": selcR})

    nc = _get_nc(debug)
    res = run_bass_kernel_spmd(nc, in_maps, list(range(N_CORES)),
                               trace=trace)
    LAST_RESULTS = res
    return np.asarray(res.results[0]["out"], np.float32)
